# revision 9
# baseline (speedup 1.0000x reference)
"""GAT-style attention-diagonal kernel for Trainium2 (Bass/Tile), 8-core SPMD.

Reference computation (per (b,t) slice, x:[N,F]):
    Q = x@Wq + bq; K = x@Wk + bk; V = x@Wv + bv
    s = Q @ K.T / sqrt(F)            # [N,N]
    a = softmax(s, axis=-1)
    out = diag(a)[:, None] * V       # only the softmax diagonal is needed

Sharding: data-parallel on the fused B*T axis (48 slices -> 6 per core),
weights replicated.

v2 dataflow (fused path, bq=bk=bv=0), mixed precision:
  - the N x N score matrix is only needed for the softmax DENOMINATOR
    (row sums of exp), which averages ~1024 terms: fp8(e4m3) inputs give
    ~0.5% denominator error.  The score matmul runs in fp8 DoubleRow
    mode (2 contraction rows per PE pass = 2x f32r throughput).
  - the diagonal s_nn (which sets output accuracy, ~exp sensitivity 1:1)
    is recomputed exactly per 128-row chunk as a [128,128] bf16 matmul
    block (XMT_bf16 x XT_bf16), diagonal extracted by identity mask.
  - everything else (x transpose, XM projection, V projection) runs in
    bf16 (same PE rate as f32r, but 1.0 vs 2.0 cy/row transposes and 2x
    DVE copy rate).  One-time M = Wq @ Wk.T eliminates the K projection.
  - error budget: bf16 X/M/V ~0.2%, fp8 denominator ~0.5%, total ~0.6%
    against the 2e-2 harness tolerance.
"""

import numpy as np

B, T, N, F = 4, 12, 1024, 512
NCORES = 8
S = (B * T) // NCORES  # 6 slices per core
P = 128
NO = N // P   # 8 row chunks per slice
FO = F // P   # 4 f chunks
GO = F // P   # 4 g chunks
MH = N // 512  # 2 halves of the scores free axis
SCALE = float(1.0 / np.sqrt(np.float32(F)))

_CACHE: dict = {}


def build_program(
    n_slices: int = S,
    repeats: int = 1,
    fused_qk: bool = True,
    use_for_i: bool = False,
):
    import concourse.bass as bass
    import concourse.tile as tile
    from concourse import bacc, mybir
    from concourse.masks import make_identity
    from contextlib import ExitStack

    f32 = mybir.dt.float32
    f32r = mybir.dt.float32r
    bf16 = mybir.dt.bfloat16
    fp8 = mybir.dt.float8e4
    DR = mybir.MatmulPerfMode.DoubleRow
    EXP = mybir.ActivationFunctionType.Exp
    COPYF = mybir.ActivationFunctionType.Identity
    AX = mybir.AxisListType.X
    OP = mybir.AluOpType

    nc = bacc.Bacc(trn_type="TRN2", target_bir_lowering=False, debug=False)
    x_d = nc.dram_tensor("x", [n_slices, N, F], f32, kind="ExternalInput").ap()
    wq_d = nc.dram_tensor("wq", [F, F], f32, kind="ExternalInput").ap()
    wk_d = nc.dram_tensor("wk", [F, F], f32, kind="ExternalInput").ap()
    wv_d = nc.dram_tensor("wv", [F, F], f32, kind="ExternalInput").ap()
    bq_d = nc.dram_tensor("bq", [F], f32, kind="ExternalInput").ap()
    bk_d = nc.dram_tensor("bk", [F], f32, kind="ExternalInput").ap()
    bv_d = nc.dram_tensor("bv", [F], f32, kind="ExternalInput").ap()
    out_d = nc.dram_tensor("out", [n_slices, N, F], f32, kind="ExternalOutput").ap()

    with tile.TileContext(nc) as tc, ExitStack() as ctx:
        consts = ctx.enter_context(tc.tile_pool(name="consts", bufs=1))
        stage = ctx.enter_context(tc.tile_pool(name="stage", bufs=1))
        xin_pool = ctx.enter_context(tc.tile_pool(name="xin", bufs=2))
        xbf_pool = ctx.enter_context(tc.tile_pool(name="xbf", bufs=2))
        xt_pool = ctx.enter_context(tc.tile_pool(name="xt", bufs=2))
        proj_pool = ctx.enter_context(tc.tile_pool(name="proj", bufs=2))
        outp = ctx.enter_context(tc.tile_pool(name="outp", bufs=3))
        dscr = ctx.enter_context(tc.tile_pool(name="dscr", bufs=2))
        stats = ctx.enter_context(tc.tile_pool(name="stats", bufs=6))
        # PSUM budget: 8 banks = pp(1tag x2) + sp(2tags x1) + tp(1tag x2)
        # + dp(1tag x2); pool size = n_tags * bufs banks
        pp = ctx.enter_context(tc.tile_pool(name="pp", bufs=2, space="PSUM"))
        sp = ctx.enter_context(tc.tile_pool(name="sp", bufs=1, space="PSUM"))
        tp = ctx.enter_context(tc.tile_pool(name="tp", bufs=2, space="PSUM"))
        dp = ctx.enter_context(tc.tile_pool(name="dp", bufs=2, space="PSUM"))

        ident = consts.tile([P, P], f32, name="ident", tag="ident")
        make_identity(nc, ident[:])
        ident_bf = consts.tile([P, P], bf16, name="ident_bf", tag="ident_bf")
        nc.vector.tensor_copy(ident_bf[:], ident[:])

        def load_and_transpose_x(s, fine_first=False):
            """x slice -> SBUF(f32) -> bf16 -> PE transpose -> XT in bf16
            and fp8.  Per-row-chunk so each transpose group starts as soon
            as its own chunk lands."""
            x_sb = xin_pool.tile([P, NO, F], f32, name="x_sb", tag="x_sb")
            x_r = x_d[s].rearrange("(no p) f -> p no f", p=P)
            for no in range(NO):
                if fine_first and no == 0:
                    for fo in range(FO):
                        nc.sync.dma_start(
                            x_sb[:, 0, fo * P : (fo + 1) * P],
                            x_r[:, 0, fo * P : (fo + 1) * P],
                        )
                    continue
                nc.sync.dma_start(x_sb[:, no : no + 1], x_r[:, no : no + 1])
            xbf_sb = xbf_pool.tile([P, NO, F], bf16, name="xbf_sb", tag="xbf_sb")
            xt_bf = xt_pool.tile([P, FO, N], bf16, name="xt_bf", tag="xt_bf")
            xt_8 = xt_pool.tile([P, FO, N], fp8, name="xt_8", tag="xt_8")
            for no in range(NO):
                nc.scalar.activation(xbf_sb[:, no, :], x_sb[:, no, :], COPYF)
                t_ps = tp.tile([P, FO, P], bf16, name="t_ps", tag="t_ps")
                for fo in range(FO):
                    nc.tensor.transpose(
                        t_ps[:, fo], xbf_sb[:, no, fo * P : (fo + 1) * P],
                        ident_bf[:],
                    )
                nc.vector.tensor_copy(xt_bf[:, :, no * P : (no + 1) * P], t_ps[:])
                nc.vector.tensor_copy(xt_8[:, :, no * P : (no + 1) * P], t_ps[:])
            return xt_bf, xt_8

        slice_list = [sl for _ in range(repeats) for sl in range(n_slices)]

        # emit slice 0's load+transpose FIRST (not in For_i mode) so the PE
        # starts work while the serialized weight-stage DMAs proceed
        xt_first = None
        if not use_for_i:
            xt_first = load_and_transpose_x(slice_list[0], fine_first=True)

        # weights staged as f32
        w_stages = {}
        for nm, wd in (("wq", wq_d), ("wk", wk_d), ("wv", wv_d)):
            w_stage = stage.tile([P, FO, F], f32, name=f"{nm}_stage", tag=f"{nm}_stage")
            nc.sync.dma_start(w_stage[:], wd.rearrange("(fo fi) g -> fi fo g", fi=P))
            w_stages[nm] = w_stage

        wv_bf = consts.tile([P, FO, F], bf16, name="wv_bf", tag="wv_bf")
        nc.vector.tensor_copy(wv_bf[:], w_stages["wv"][:])

        # biases (general path only)
        bq_sb = consts.tile([P, GO], f32, name="bq_sb", tag="bq_sb")
        nc.sync.dma_start(bq_sb[:], bq_d.rearrange("(go gi) -> gi go", gi=P))
        bk_sb = consts.tile([P, GO], f32, name="bk_sb", tag="bk_sb")
        nc.sync.dma_start(bk_sb[:], bk_d.rearrange("(go gi) -> gi go", gi=P))
        bv_bc = consts.tile([P, F], f32, name="bv_bc", tag="bv_bc")
        nc.sync.dma_start(bv_bc[:], bv_d.unsqueeze(0).to_broadcast((P, F)))

        if fused_qk:
            # one-time M = Wq @ Wk.T, stored bf16 like a weight [ai, ao, b].
            # Transposes run in bf16 so they share the tp pool's t_ps tag.
            wt_sbs = {}
            for nm in ("wq", "wk"):
                w_bf = stage.tile([P, FO, F], bf16, name=f"{nm}_bfs", tag=f"{nm}_bfs")
                nc.vector.tensor_copy(w_bf[:], w_stages[nm][:])
                wt_sb = consts.tile([P, FO, F], bf16, name=f"{nm}t_sb", tag=f"{nm}t_sb")
                for ao in range(FO):
                    t_ps = tp.tile([P, FO, P], bf16, name="t_ps", tag="t_ps")
                    for co in range(FO):
                        nc.tensor.transpose(
                            t_ps[:, co],
                            w_bf[:, ao, co * P : (co + 1) * P],
                            ident_bf[:],
                        )
                    nc.vector.tensor_copy(wt_sb[:, :, ao * P : (ao + 1) * P], t_ps[:])
                wt_sbs[nm] = wt_sb
            m_bf = consts.tile([P, FO, F], bf16, name="m_bf", tag="m_bf")
            for ao in range(FO):
                ps = pp.tile([P, F], f32, name="ps_proj", tag="ps_proj")
                for co in range(FO):
                    nc.tensor.matmul(
                        ps[:],
                        wt_sbs["wq"][:, co, ao * P : (ao + 1) * P],
                        wt_sbs["wk"][:, co, :],
                        start=(co == 0),
                        stop=(co == FO - 1),
                    )
                nc.vector.tensor_copy(m_bf[:, ao, :], ps[:])
        else:
            w_sbs = {}
            for nm in ("wq", "wk"):
                w_sb = consts.tile([P, FO, F], bf16, name=f"{nm}_sb", tag=f"{nm}_sb")
                nc.vector.tensor_copy(w_sb[:], w_stages[nm][:])
                w_sbs[nm] = w_sb

        def emit_slice_fused(s, xt_pair):
            xt_bf, xt_8 = (
                xt_pair if xt_pair is not None else load_and_transpose_x(s)
            )

            # XMT[b, n] = sum_a M[a, b] * XT[a, n]  -> bf16 + fp8 copies
            xmt_bf = proj_pool.tile([P, GO, N], bf16, name="xmt_bf", tag="xmt_bf")
            xmt_8 = proj_pool.tile([P, GO, N], fp8, name="xmt_8", tag="xmt_8")
            for go in range(GO):
                for nh in range(MH):
                    ps = pp.tile([P, 512], f32, name="ps_proj", tag="ps_proj")
                    for fo in range(FO):
                        nc.tensor.matmul(
                            ps[:],
                            m_bf[:, fo, go * P : (go + 1) * P],
                            xt_bf[:, fo, nh * 512 : (nh + 1) * 512],
                            start=(fo == 0),
                            stop=(fo == FO - 1),
                        )
                    nc.scalar.activation(
                        xmt_bf[:, go, nh * 512 : (nh + 1) * 512], ps[:], COPYF
                    )
                    nc.vector.tensor_copy(
                        xmt_8[:, go, nh * 512 : (nh + 1) * 512], ps[:]
                    )

            for no in range(NO):
                nsl = slice(no * P, (no + 1) * P)
                # scores row-chunk in fp8 DoubleRow: 2 k-pair passes per bank
                ps0 = sp.tile([P, 512], f32, name="ps_s0", tag="ps_s0")
                ps1 = sp.tile([P, 512], f32, name="ps_s1", tag="ps_s1")
                for j in range(2):
                    ksl = slice(2 * j, 2 * j + 2)
                    lhsT = xmt_8[:, ksl, nsl]
                    nc.tensor.matmul(
                        ps0[:], lhsT, xt_8[:, ksl, 0:512],
                        start=(j == 0), stop=(j == 1), perf_mode=DR,
                    )
                    nc.tensor.matmul(
                        ps1[:], lhsT, xt_8[:, ksl, 512:1024],
                        start=(j == 0), stop=(j == 1), perf_mode=DR,
                    )
                # exact-ish diagonal block (bf16)
                dps = dp.tile([P, P], f32, name="dps", tag="dps")
                for go in range(GO):
                    nc.tensor.matmul(
                        dps[:],
                        xmt_bf[:, go, nsl],
                        xt_bf[:, go, nsl],
                        start=(go == 0),
                        stop=(go == GO - 1),
                    )
                # V row-chunk (bf16), scaled straight from PSUM at the end
                ps_v = pp.tile([P, F], f32, name="ps_proj", tag="ps_proj")
                for fo in range(FO):
                    nc.tensor.matmul(
                        ps_v[:],
                        xt_bf[:, fo, nsl],
                        wv_bf[:, fo, :],
                        start=(fo == 0),
                        stop=(fo == FO - 1),
                    )

                # denominator: exp fused with row-sum on ScalarE (scores are
                # ~N(0,1) after scale; no max-subtraction needed)
                s0 = stats.tile([P, 1], f32, name="s0", tag="s0")
                s1 = stats.tile([P, 1], f32, name="s1", tag="s1")
                nc.scalar.activation(ps0[:], ps0[:], EXP, scale=SCALE, accum_out=s0[:])
                nc.scalar.activation(ps1[:], ps1[:], EXP, scale=SCALE, accum_out=s1[:])

                # diagonal of the exact block via identity mask + reduce
                dblk = dscr.tile([P, P], f32, name="dblk", tag="dblk")
                nc.vector.tensor_mul(dblk[:], dps[:], ident[:])
                snn = stats.tile([P, 1], f32, name="snn", tag="snn")
                nc.vector.tensor_reduce(snn[:], dblk[:], axis=AX, op=OP.add)
                esnn = stats.tile([P, 1], f32, name="esnn", tag="esnn")
                nc.scalar.activation(esnn[:], snn[:], EXP, scale=SCALE)

                ssum = stats.tile([P, 1], f32, name="ssum", tag="ssum")
                nc.vector.tensor_add(ssum[:], s0[:], s1[:])
                rec = stats.tile([P, 1], f32, name="rec", tag="rec")
                nc.vector.reciprocal(rec[:], ssum[:])
                dval = stats.tile([P, 1], f32, name="dval", tag="dval")
                nc.vector.tensor_mul(dval[:], esnn[:], rec[:])

                ot = outp.tile([P, F], f32, name="ot", tag="ot")
                nc.scalar.activation(ot[:], ps_v[:], COPYF, scale=dval[:])
                nc.sync.dma_start(out_d[s, nsl, :], ot[:])

        def emit_slice_general(s):
            """f32r path with biases (reference-faithful Q/K projections)."""
            xt_bf, _ = load_and_transpose_x(s)
            # f32r view of XT via an extra copy (bf16 source is fine since
            # the general path is only a correctness fallback)
            v_sb = proj_pool.tile([P, NO, F], f32, name="v_sb", tag="v_sb")
            qt_sb = proj_pool.tile([P, GO, N], bf16, name="qt_sb", tag="qt_sb")
            kt_sb = proj_pool.tile([P, GO, N], bf16, name="kt_sb", tag="kt_sb")
            for w_sb, b_sb, dst in (
                (w_sbs["wq"], bq_sb, qt_sb),
                (w_sbs["wk"], bk_sb, kt_sb),
            ):
                # dst[g, n] = sum_f W[f, g] XT[f, n] + b[g]
                for go in range(GO):
                    for nh in range(MH):
                        ps = pp.tile([P, 512], f32, name="ps_proj", tag="ps_proj")
                        for fo in range(FO):
                            nc.tensor.matmul(
                                ps[:],
                                w_sb[:, fo, go * P : (go + 1) * P],
                                xt_bf[:, fo, nh * 512 : (nh + 1) * 512],
                                start=(fo == 0),
                                stop=(fo == FO - 1),
                            )
                        nc.vector.tensor_scalar_add(
                            dst[:, go, nh * 512 : (nh + 1) * 512],
                            ps[:],
                            b_sb[:, go : go + 1],
                        )
            for no in range(NO):
                ps = pp.tile([P, F], f32, name="ps_proj", tag="ps_proj")
                for fo in range(FO):
                    nc.tensor.matmul(
                        ps[:],
                        xt_bf[:, fo, no * P : (no + 1) * P],
                        wv_bf[:, fo, :],
                        start=(fo == 0),
                        stop=(fo == FO - 1),
                    )
                nc.vector.tensor_add(v_sb[:, no, :], ps[:], bv_bc[:])

            for no in range(NO):
                ps0 = sp.tile([P, 512], f32, name="ps_s0", tag="ps_s0")
                ps1 = sp.tile([P, 512], f32, name="ps_s1", tag="ps_s1")
                for go in range(GO):
                    lhsT = qt_sb[:, go, no * P : (no + 1) * P]
                    nc.tensor.matmul(
                        ps0[:], lhsT, kt_sb[:, go, 0:512],
                        start=(go == 0), stop=(go == GO - 1),
                    )
                    nc.tensor.matmul(
                        ps1[:], lhsT, kt_sb[:, go, 512:1024],
                        start=(go == 0), stop=(go == GO - 1),
                    )
                s0 = stats.tile([P, 1], f32, name="s0", tag="s0")
                s1 = stats.tile([P, 1], f32, name="s1", tag="s1")
                nc.scalar.activation(ps0[:], ps0[:], EXP, scale=SCALE, accum_out=s0[:])
                nc.scalar.activation(ps1[:], ps1[:], EXP, scale=SCALE, accum_out=s1[:])
                bank, off = divmod(no * P, 512)
                psd = ps0 if bank == 0 else ps1
                dblk = dscr.tile([P, P], f32, name="dblk", tag="dblk")
                nc.vector.tensor_mul(dblk[:], psd[:, off : off + P], ident[:])
                snn = stats.tile([P, 1], f32, name="snn", tag="snn")
                nc.vector.tensor_reduce(snn[:], dblk[:], axis=AX, op=OP.add)
                ssum = stats.tile([P, 1], f32, name="ssum", tag="ssum")
                nc.vector.tensor_add(ssum[:], s0[:], s1[:])
                rec = stats.tile([P, 1], f32, name="rec", tag="rec")
                nc.vector.reciprocal(rec[:], ssum[:])
                dval = stats.tile([P, 1], f32, name="dval", tag="dval")
                nc.vector.tensor_mul(dval[:], snn[:], rec[:])
                ot = outp.tile([P, F], f32, name="ot", tag="ot")
                nc.vector.tensor_scalar_mul(ot[:], v_sb[:, no, :], dval[:])
                nc.sync.dma_start(out_d[s, no * P : (no + 1) * P, :], ot[:])

        if use_for_i:
            with tc.For_i(0, repeats) as _i:
                for s in range(n_slices):
                    if fused_qk:
                        emit_slice_fused(s, None)
                    else:
                        emit_slice_general(s)
        else:
            for i, s in enumerate(slice_list):
                if fused_qk:
                    emit_slice_fused(s, xt_first if i == 0 else None)
                else:
                    emit_slice_general(s)

    nc.compile()
    return nc


def _get_runner(fused: bool):
    """Build the Bass program once and wrap it in a cached jitted shard_map
    dispatcher (mirrors bass2jax.run_bass_via_pjrt, minus donation so the
    pre-zeroed output operands can be reused across calls — this kernel
    writes every output element)."""
    key = ("runner", fused)
    if key in _CACHE:
        return _CACHE[key]

    import jax
    from jax.experimental.shard_map import shard_map
    from jax.sharding import Mesh, NamedSharding, PartitionSpec
    from concourse import mybir
    from concourse.bass2jax import (
        _bass_exec_p,
        install_neuronx_cc_hook,
        partition_id_tensor,
    )

    nc = build_program(S, fused_qk=fused)
    install_neuronx_cc_hook()
    partition_name = nc.partition_id_tensor.name if nc.partition_id_tensor else None

    in_names, out_names, out_avals, zero_outs = [], [], [], []
    for alloc in nc.m.functions[0].allocations:
        if not isinstance(alloc, mybir.MemoryLocationSet):
            continue
        name = alloc.memorylocations[0].name
        if alloc.kind == "ExternalInput":
            if name != partition_name:
                in_names.append(name)
        elif alloc.kind == "ExternalOutput":
            shape = tuple(alloc.tensor_shape)
            np_dt = mybir.dt.np(alloc.dtype)
            out_avals.append(jax.core.ShapedArray(shape, np_dt))
            out_names.append(name)
            zero_outs.append(np.zeros(shape, np_dt))

    n_params = len(in_names)
    all_in_names = list(in_names) + list(out_names)
    if partition_name is not None:
        all_in_names.append(partition_name)

    def _body(*args):
        operands = list(args)
        if partition_name is not None:
            operands.append(partition_id_tensor())
        outs = _bass_exec_p.bind(
            *operands,
            out_avals=tuple(out_avals),
            in_names=tuple(all_in_names),
            out_names=tuple(out_names),
            lowering_input_output_aliases=(),
            sim_require_finite=True,
            sim_require_nnan=True,
            nc=nc,
        )
        return tuple(outs)

    devices = jax.devices()[:NCORES]
    mesh = Mesh(np.asarray(devices), ("core",))
    n_outs = len(out_names)
    fn = jax.jit(
        shard_map(
            _body,
            mesh=mesh,
            in_specs=(PartitionSpec("core"),) * (n_params + n_outs),
            out_specs=(PartitionSpec("core"),) * n_outs,
            check_rep=False,
        ),
        keep_unused=True,
    )
    sharding = NamedSharding(mesh, PartitionSpec("core"))
    concat_zeros = [
        jax.device_put(
            np.zeros((NCORES * z.shape[0], *z.shape[1:]), z.dtype), sharding
        )
        for z in zero_outs
    ]
    runner = {
        "fn": fn,
        "in_names": in_names,
        "out_names": out_names,
        "zeros": concat_zeros,
        "sharding": sharding,
    }
    _CACHE[key] = runner
    return runner


def kernel(x, Wq, bq, Wk, bk, Wv, bv):
    import jax

    x = np.ascontiguousarray(np.asarray(x, dtype=np.float32))
    shards = x.reshape(B * T, N, F).reshape(NCORES, S, N, F)

    bq = np.ascontiguousarray(np.asarray(bq, dtype=np.float32))
    bk = np.ascontiguousarray(np.asarray(bk, dtype=np.float32))
    bv_arr = np.ascontiguousarray(np.asarray(bv, dtype=np.float32))
    # the fused path assumes zero biases (scores = X (Wq Wk^T) X^T and V
    # scaled straight from PSUM); fall back to the general path otherwise
    fused = bool(not bq.any() and not bk.any() and not bv_arr.any())

    runner = _get_runner(fused)

    per_core = {
        "x": shards.reshape(NCORES * S, N, F),
        "wq": np.tile(np.asarray(Wq, np.float32)[None], (NCORES, 1, 1)).reshape(
            NCORES * F, F
        ),
        "wk": np.tile(np.asarray(Wk, np.float32)[None], (NCORES, 1, 1)).reshape(
            NCORES * F, F
        ),
        "wv": np.tile(np.asarray(Wv, np.float32)[None], (NCORES, 1, 1)).reshape(
            NCORES * F, F
        ),
        "bq": np.tile(bq, NCORES),
        "bk": np.tile(bk, NCORES),
        "bv": np.tile(bv_arr, NCORES),
    }
    def _run(r):
        args = [
            jax.device_put(np.ascontiguousarray(per_core[nm]), r["sharding"])
            for nm in r["in_names"]
        ]
        outs = r["fn"](*args, *r["zeros"])
        return np.asarray(outs[r["out_names"].index("out")])

    try:
        out = _run(runner)
    except Exception:
        # stale cached executable/buffers (e.g. device session reset
        # between calls): rebuild once and retry
        _CACHE.pop(("runner", fused), None)
        out = _run(_get_runner(fused))
    return out.reshape(B, T, N, F)


# revision 36
# speedup vs baseline: 1.1147x; 1.1147x over previous
"""GAT-style attention-diagonal kernel for Trainium2 (Bass/Tile), 8-core SPMD.

Reference computation (per (b,t) slice, x:[N,F]):
    Q = x@Wq + bq; K = x@Wk + bk; V = x@Wv + bv
    s = Q @ K.T / sqrt(F)            # [N,N]
    a = softmax(s, axis=-1)
    out = diag(a)[:, None] * V       # only the softmax diagonal is needed

Sharding: data-parallel on the fused B*T axis (48 slices -> 6 per core),
weights replicated.

v4 dataflow (fused path, bq=bk=bv=0), mixed precision:
  - host supplies x TRANSPOSED per slice as [fi, fo, n] in BOTH bf16 and
    fp8(e4m3) (pure layout/dtype prep, like the sharding reshape); this
    removes the on-device transposes, the f32->bf16 pass and both
    PSUM->SBUF transpose copies -- the dominant cross-engine feed chain --
    and frees 2 PSUM banks so the score accumulators double-buffer.
  - the N x N score matrix is only needed for the softmax DENOMINATOR
    (row sums of exp, ~1024 terms): fp8 inputs give ~0.5% denominator
    error.  The score matmul runs in fp8 DoubleRow mode (2 contraction
    rows per PE pass = 2x bf16 throughput).
  - the diagonal s_nn (which sets output accuracy) is recomputed exactly
    per 128-row chunk as a [128,128] bf16 matmul block, diagonal
    extracted by identity mask on DVE.
  - V projection and the one-time-fused XM projection (M = Wq @ Wk.T,
    eliminating the K projection) run in bf16.
  - error budget: bf16 X/M/V ~0.2%, fp8 denominator ~0.5% against the
    2e-2 harness tolerance.
"""

import numpy as np

B, T, N, F = 4, 12, 1024, 512
NCORES = 8
S = (B * T) // NCORES  # 6 slices per core
P = 128
NO = N // P   # 8 row chunks per slice
FO = F // P   # 4 f chunks
GO = F // P   # 4 g chunks
MH = N // 512  # 2 halves of the scores free axis
SCALE = float(1.0 / np.sqrt(np.float32(F)))

_CACHE: dict = {}


def _np_dtypes():
    from concourse import mybir

    return mybir.dt.np(mybir.dt.bfloat16), mybir.dt.np(mybir.dt.float8e4)


def prep_xt(x):
    """Full x [B,T,N,F] (or [BT,N,F]) -> per-slice transposed [BT,P,FO,N]
    in bf16 and fp8e4.  Pure layout/dtype host prep."""
    bf16_t, fp8_t = _np_dtypes()
    x = np.asarray(x, np.float32).reshape(B * T, N, F)
    # A[s, fi, fo, n] = x[s, n, fo*P + fi]
    x4 = np.ascontiguousarray(x.reshape(B * T, N, FO, P).transpose(0, 3, 2, 1))
    return x4.astype(bf16_t), x4.astype(fp8_t)


def build_program(
    n_slices: int = S,
    repeats: int = 1,
    fused_qk: bool = True,
    use_for_i: bool = False,
    pe_only: bool = False,
    lvl: int = 3,
):
    if pe_only:
        lvl = 0
    import concourse.bass as bass
    import concourse.tile as tile
    from concourse import bacc, mybir
    from concourse.masks import make_identity
    from contextlib import ExitStack

    f32 = mybir.dt.float32
    bf16 = mybir.dt.bfloat16
    fp8 = mybir.dt.float8e4
    DR = mybir.MatmulPerfMode.DoubleRow
    EXP = mybir.ActivationFunctionType.Exp
    COPYF = mybir.ActivationFunctionType.Identity
    AX = mybir.AxisListType.X
    OP = mybir.AluOpType

    nc = bacc.Bacc(trn_type="TRN2", target_bir_lowering=False, debug=False)
    xtbf_d = nc.dram_tensor(
        "xtbf", [n_slices, P, FO, N], bf16, kind="ExternalInput"
    ).ap()
    xt8_d = nc.dram_tensor(
        "xt8", [n_slices, P, FO, N], fp8, kind="ExternalInput"
    ).ap()
    wq_d = nc.dram_tensor("wq", [F, F], f32, kind="ExternalInput").ap()
    wk_d = nc.dram_tensor("wk", [F, F], f32, kind="ExternalInput").ap()
    wv_d = nc.dram_tensor("wv", [F, F], f32, kind="ExternalInput").ap()
    bq_d = nc.dram_tensor("bq", [F], f32, kind="ExternalInput").ap()
    bk_d = nc.dram_tensor("bk", [F], f32, kind="ExternalInput").ap()
    bv_d = nc.dram_tensor("bv", [F], f32, kind="ExternalInput").ap()
    out_d = nc.dram_tensor("out", [n_slices, N, F], f32, kind="ExternalOutput").ap()

    with tile.TileContext(nc) as tc, ExitStack() as ctx:
        consts = ctx.enter_context(tc.tile_pool(name="consts", bufs=1))
        stage = ctx.enter_context(tc.tile_pool(name="stage", bufs=1))
        xt_pool = ctx.enter_context(tc.tile_pool(name="xt", bufs=2))
        proj_pool = ctx.enter_context(tc.tile_pool(name="proj", bufs=2))
        outp = ctx.enter_context(tc.tile_pool(name="outp", bufs=3))
        dscr = ctx.enter_context(tc.tile_pool(name="dscr", bufs=2))
        stats = ctx.enter_context(tc.tile_pool(name="stats", bufs=6))
        # PSUM budget: 8 banks = pp(1tag x2) + sp(2tags x2) + dp(1tag x2)
        pp = ctx.enter_context(tc.tile_pool(name="pp", bufs=2, space="PSUM"))
        sp = ctx.enter_context(tc.tile_pool(name="sp", bufs=2, space="PSUM"))
        dp = ctx.enter_context(tc.tile_pool(name="dp", bufs=2, space="PSUM"))

        ident = consts.tile([P, P], f32, name="ident", tag="ident")
        make_identity(nc, ident[:])

        g = {}
        if lvl == 0:
            for nm, shape, dt_ in (
                ("g_xmtbf", [P, GO, N], bf16),
                ("g_xmt8", [P, GO, N], fp8),
            ):
                t = consts.tile(shape, dt_, name=nm, tag=nm)
                nc.vector.memset(t[:], 0)
                g[nm] = t

        def load_xt(s):
            """DMA the host-pretransposed slice: bf16 + fp8, per-fo chunks
            so the XM accumulation starts as soon as fo=0 lands."""
            xt_bf = xt_pool.tile([P, FO, N], bf16, name="xt_bf", tag="xt_bf")
            xt_8 = xt_pool.tile([P, FO, N], fp8, name="xt_8", tag="xt_8")
            for fo in range(0, FO, 2):
                nc.sync.dma_start(xt_bf[:, fo : fo + 2], xtbf_d[s, :, fo : fo + 2])
            nc.sync.dma_start(xt_8[:], xt8_d[s])
            return xt_bf, xt_8

        slice_list = [sl for _ in range(repeats) for sl in range(n_slices)]

        # emit slice 0's loads first so compute starts under weight staging
        xt_first = None
        if not use_for_i:
            xt_first = load_xt(slice_list[0])

        # weights staged as f32
        w_stages = {}
        for nm, wd in (("wq", wq_d), ("wk", wk_d), ("wv", wv_d)):
            w_stage = stage.tile([P, FO, F], f32, name=f"{nm}_stage", tag=f"{nm}_stage")
            nc.sync.dma_start(w_stage[:], wd.rearrange("(fo fi) g -> fi fo g", fi=P))
            w_stages[nm] = w_stage

        wv_bf = consts.tile([P, FO, F], bf16, name="wv_bf", tag="wv_bf")
        nc.vector.tensor_copy(wv_bf[:], w_stages["wv"][:])

        # biases (general path only)
        bq_sb = consts.tile([P, GO], f32, name="bq_sb", tag="bq_sb")
        nc.sync.dma_start(bq_sb[:], bq_d.rearrange("(go gi) -> gi go", gi=P))
        bk_sb = consts.tile([P, GO], f32, name="bk_sb", tag="bk_sb")
        nc.sync.dma_start(bk_sb[:], bk_d.rearrange("(go gi) -> gi go", gi=P))
        bv_bc = consts.tile([P, F], f32, name="bv_bc", tag="bv_bc")
        nc.sync.dma_start(bv_bc[:], bv_d.unsqueeze(0).to_broadcast((P, F)))

        if fused_qk:
            # one-time M = Wq @ Wk.T, stored bf16 like a weight [ai, ao, b].
            # f32 PE transposes through the dp pool's [P, P] bank.
            wt_sbs = {}
            for nm in ("wq", "wk"):
                wt_sb = consts.tile([P, FO, F], bf16, name=f"{nm}t_sb", tag=f"{nm}t_sb")
                for ao in range(FO):
                    for co in range(FO):
                        t_ps = dp.tile([P, P], f32, name="dps", tag="dps")
                        nc.tensor.transpose(
                            t_ps[:],
                            w_stages[nm][:, ao, co * P : (co + 1) * P],
                            ident[:],
                        )
                        nc.vector.tensor_copy(
                            wt_sb[:, co, ao * P : (ao + 1) * P], t_ps[:]
                        )
                wt_sbs[nm] = wt_sb
            m_bf = consts.tile([P, FO, F], bf16, name="m_bf", tag="m_bf")
            for ao in range(FO):
                ps = pp.tile([P, F], f32, name="ps_proj", tag="ps_proj")
                for co in range(FO):
                    nc.tensor.matmul(
                        ps[:],
                        wt_sbs["wq"][:, co, ao * P : (ao + 1) * P],
                        wt_sbs["wk"][:, co, :],
                        start=(co == 0),
                        stop=(co == FO - 1),
                    )
                nc.vector.tensor_copy(m_bf[:, ao, :], ps[:])
        else:
            w_sbs = {}
            for nm in ("wq", "wk"):
                w_sb = consts.tile([P, FO, F], bf16, name=f"{nm}_sb", tag=f"{nm}_sb")
                nc.vector.tensor_copy(w_sb[:], w_stages[nm][:])
                w_sbs[nm] = w_sb

        def emit_slice_fused(s, xt_pair):
            xt_bf, xt_8 = xt_pair if xt_pair is not None else load_xt(s)

            # XMT[b, n] = sum_a M[a, b] * XT[a, n]  -> bf16 + fp8 copies
            if lvl == 0:
                xmt_bf, xmt_8 = g["g_xmtbf"], g["g_xmt8"]
            else:
                xmt_bf = proj_pool.tile([P, GO, N], bf16, name="xmt_bf", tag="xmt_bf")
                xmt_8 = proj_pool.tile([P, GO, N], fp8, name="xmt_8", tag="xmt_8")
            for go in range(GO):
                for nh in range(MH):
                    hsl = slice(nh * 512, (nh + 1) * 512)
                    ps = pp.tile([P, 512], f32, name="ps_proj", tag="ps_proj")
                    for fo in range(FO):
                        nc.tensor.matmul(
                            ps[:],
                            m_bf[:, fo, go * P : (go + 1) * P],
                            xt_bf[:, fo, hsl],
                            start=(fo == 0),
                            stop=(fo == FO - 1),
                        )
                    if lvl >= 1:
                        nc.scalar.activation(xmt_bf[:, go, hsl], ps[:], COPYF)
                        nc.vector.tensor_copy(xmt_8[:, go, hsl], ps[:])

            out_r = out_d[s].rearrange("(g p) f -> p g f", p=P)

            def finish_chunk(m, ps_v, s0, s1, snn, ot2):
                # tail for chunk m, emitted during chunk m+1: keeps the
                # dval chain off the next chunk's critical path
                s01 = stats.tile([P, 1], f32, name="s01", tag="s01")
                nc.vector.tensor_add(s01[:], s0[:], s1[:])
                esnn = stats.tile([P, 1], f32, name="esnn", tag="esnn")
                nc.scalar.activation(esnn[:], snn[:], EXP, scale=SCALE)
                rec = stats.tile([P, 1], f32, name="rec", tag="rec")
                nc.vector.reciprocal(rec[:], s01[:])
                dval = stats.tile([P, 1], f32, name="dval", tag="dval")
                nc.vector.tensor_mul(dval[:], esnn[:], rec[:])
                nc.vector.tensor_scalar_mul(ot2[:, m % 2], ps_v[:], dval[:])
                if m % 2 == 1:
                    nc.sync.dma_start(out_r[:, m - 1 : m + 1], ot2[:])

            prev = None
            ot2 = None
            for no in range(NO):
                nsl = slice(no * P, (no + 1) * P)
                # exact-ish diagonal block (bf16) and V first: they cover
                # the xmt_8 feed latency at the XMT/chunk-loop boundary
                dps = dp.tile([P, P], f32, name="dps", tag="dps")
                for go in range(GO):
                    nc.tensor.matmul(
                        dps[:],
                        xmt_bf[:, go, nsl],
                        xt_bf[:, go, nsl],
                        start=(go == 0),
                        stop=(go == GO - 1),
                    )
                # V row-chunk (bf16), scaled straight from PSUM in the tail
                ps_v = pp.tile([P, F], f32, name="ps_proj", tag="ps_proj")
                for fo in range(FO):
                    nc.tensor.matmul(
                        ps_v[:],
                        xt_bf[:, fo, nsl],
                        wv_bf[:, fo, :],
                        start=(fo == 0),
                        stop=(fo == FO - 1),
                    )
                # scores row-chunk in fp8 DoubleRow: 2 k-pair passes per bank
                ps0 = sp.tile([P, 512], f32, name="ps_s0", tag="ps_s0")
                ps1 = sp.tile([P, 512], f32, name="ps_s1", tag="ps_s1")
                for j in range(2):
                    ksl = slice(2 * j, 2 * j + 2)
                    lhsT = xmt_8[:, ksl, nsl]
                    nc.tensor.matmul(
                        ps0[:], lhsT, xt_8[:, ksl, 0:512],
                        start=(j == 0), stop=(j == 1), perf_mode=DR,
                    )
                    nc.tensor.matmul(
                        ps1[:], lhsT, xt_8[:, ksl, 512:1024],
                        start=(j == 0), stop=(j == 1), perf_mode=DR,
                    )

                if lvl < 2:
                    continue
                s0 = stats.tile([P, 1], f32, name="s0", tag="s0")
                s1 = stats.tile([P, 1], f32, name="s1", tag="s1")
                nc.scalar.activation(ps0[:], ps0[:], EXP, scale=SCALE, accum_out=s0[:])
                nc.scalar.activation(ps1[:], ps1[:], EXP, scale=SCALE, accum_out=s1[:])
                if lvl < 3:
                    continue
                dblk = dscr.tile([P, P], f32, name="dblk", tag="dblk")
                nc.vector.tensor_mul(dblk[:], dps[:], ident[:])
                snn = stats.tile([P, 1], f32, name="snn", tag="snn")
                nc.vector.tensor_reduce(snn[:], dblk[:], axis=AX, op=OP.add)

                if no % 2 == 0:
                    ot2 = outp.tile([P, 2, F], f32, name="ot2", tag="ot2")
                if prev is not None:
                    finish_chunk(*prev)
                prev = (no, ps_v, s0, s1, snn, ot2)
            if lvl >= 3:
                finish_chunk(*prev)

        def emit_slice_general(s):
            """bf16 path with biases (reference-faithful Q/K projections)."""
            xt_bf, _ = load_xt(s)
            v_sb = proj_pool.tile([P, NO, F], f32, name="v_sb", tag="v_sb")
            qt_sb = proj_pool.tile([P, GO, N], bf16, name="qt_sb", tag="qt_sb")
            kt_sb = proj_pool.tile([P, GO, N], bf16, name="kt_sb", tag="kt_sb")
            for w_sb, b_sb, dst in (
                (w_sbs["wq"], bq_sb, qt_sb),
                (w_sbs["wk"], bk_sb, kt_sb),
            ):
                for go in range(GO):
                    for nh in range(MH):
                        ps = pp.tile([P, 512], f32, name="ps_proj", tag="ps_proj")
                        for fo in range(FO):
                            nc.tensor.matmul(
                                ps[:],
                                w_sb[:, fo, go * P : (go + 1) * P],
                                xt_bf[:, fo, nh * 512 : (nh + 1) * 512],
                                start=(fo == 0),
                                stop=(fo == FO - 1),
                            )
                        nc.vector.tensor_scalar_add(
                            dst[:, go, nh * 512 : (nh + 1) * 512],
                            ps[:],
                            b_sb[:, go : go + 1],
                        )
            for no in range(NO):
                ps = pp.tile([P, F], f32, name="ps_proj", tag="ps_proj")
                for fo in range(FO):
                    nc.tensor.matmul(
                        ps[:],
                        xt_bf[:, fo, no * P : (no + 1) * P],
                        wv_bf[:, fo, :],
                        start=(fo == 0),
                        stop=(fo == FO - 1),
                    )
                nc.vector.tensor_add(v_sb[:, no, :], ps[:], bv_bc[:])

            for no in range(NO):
                ps0 = sp.tile([P, 512], f32, name="ps_s0", tag="ps_s0")
                ps1 = sp.tile([P, 512], f32, name="ps_s1", tag="ps_s1")
                for go in range(GO):
                    lhsT = qt_sb[:, go, no * P : (no + 1) * P]
                    nc.tensor.matmul(
                        ps0[:], lhsT, kt_sb[:, go, 0:512],
                        start=(go == 0), stop=(go == GO - 1),
                    )
                    nc.tensor.matmul(
                        ps1[:], lhsT, kt_sb[:, go, 512:1024],
                        start=(go == 0), stop=(go == GO - 1),
                    )
                s0 = stats.tile([P, 1], f32, name="s0", tag="s0")
                s1 = stats.tile([P, 1], f32, name="s1", tag="s1")
                nc.scalar.activation(ps0[:], ps0[:], EXP, scale=SCALE, accum_out=s0[:])
                nc.scalar.activation(ps1[:], ps1[:], EXP, scale=SCALE, accum_out=s1[:])
                bank, off = divmod(no * P, 512)
                psd = ps0 if bank == 0 else ps1
                dblk = dscr.tile([P, P], f32, name="dblk", tag="dblk")
                nc.vector.tensor_mul(dblk[:], psd[:, off : off + P], ident[:])
                snn = stats.tile([P, 1], f32, name="snn", tag="snn")
                nc.vector.tensor_reduce(snn[:], dblk[:], axis=AX, op=OP.add)
                ssum = stats.tile([P, 1], f32, name="ssum", tag="ssum")
                nc.vector.tensor_add(ssum[:], s0[:], s1[:])
                rec = stats.tile([P, 1], f32, name="rec", tag="rec")
                nc.vector.reciprocal(rec[:], ssum[:])
                dval = stats.tile([P, 1], f32, name="dval", tag="dval")
                nc.vector.tensor_mul(dval[:], snn[:], rec[:])
                ot = outp.tile([P, F], f32, name="ot", tag="ot")
                nc.vector.tensor_scalar_mul(ot[:], v_sb[:, no, :], dval[:])
                nc.sync.dma_start(out_d[s, no * P : (no + 1) * P, :], ot[:])

        if use_for_i:
            with tc.For_i(0, repeats) as _i:
                for s in range(n_slices):
                    if fused_qk:
                        emit_slice_fused(s, None)
                    else:
                        emit_slice_general(s)
        else:
            for i, s in enumerate(slice_list):
                if fused_qk:
                    emit_slice_fused(s, xt_first if i == 0 else None)
                else:
                    emit_slice_general(s)

    nc.compile()
    return nc


def make_in_maps(x, Wq, bq, Wk, bk, Wv, bv):
    """Per-core input dicts for the Bass program (host does the transpose
    + bf16/fp8 casts)."""
    xtbf, xt8 = prep_xt(x)
    xtbf = xtbf.reshape(NCORES, S, P, FO, N)
    xt8 = xt8.reshape(NCORES, S, P, FO, N)
    common = {
        "wq": np.asarray(Wq, np.float32),
        "wk": np.asarray(Wk, np.float32),
        "wv": np.asarray(Wv, np.float32),
        "bq": np.asarray(bq, np.float32),
        "bk": np.asarray(bk, np.float32),
        "bv": np.asarray(bv, np.float32),
    }
    return [dict(common, xtbf=xtbf[c], xt8=xt8[c]) for c in range(NCORES)]


def _get_runner(fused: bool):
    """Build the Bass program once and wrap it in a cached jitted shard_map
    dispatcher (mirrors bass2jax.run_bass_via_pjrt, minus donation so the
    pre-zeroed output operands can be reused across calls — this kernel
    writes every output element)."""
    key = ("runner", fused)
    if key in _CACHE:
        return _CACHE[key]

    import jax
    from jax.experimental.shard_map import shard_map
    from jax.sharding import Mesh, NamedSharding, PartitionSpec
    from concourse import mybir
    from concourse.bass2jax import (
        _bass_exec_p,
        install_neuronx_cc_hook,
        partition_id_tensor,
    )

    nc = build_program(S, fused_qk=fused)
    install_neuronx_cc_hook()
    partition_name = nc.partition_id_tensor.name if nc.partition_id_tensor else None

    in_names, out_names, out_avals, zero_outs = [], [], [], []
    for alloc in nc.m.functions[0].allocations:
        if not isinstance(alloc, mybir.MemoryLocationSet):
            continue
        name = alloc.memorylocations[0].name
        if alloc.kind == "ExternalInput":
            if name != partition_name:
                in_names.append(name)
        elif alloc.kind == "ExternalOutput":
            shape = tuple(alloc.tensor_shape)
            np_dt = mybir.dt.np(alloc.dtype)
            out_avals.append(jax.core.ShapedArray(shape, np_dt))
            out_names.append(name)
            zero_outs.append(np.zeros(shape, np_dt))

    n_params = len(in_names)
    all_in_names = list(in_names) + list(out_names)
    if partition_name is not None:
        all_in_names.append(partition_name)

    def _body(*args):
        operands = list(args)
        if partition_name is not None:
            operands.append(partition_id_tensor())
        outs = _bass_exec_p.bind(
            *operands,
            out_avals=tuple(out_avals),
            in_names=tuple(all_in_names),
            out_names=tuple(out_names),
            lowering_input_output_aliases=(),
            sim_require_finite=True,
            sim_require_nnan=True,
            nc=nc,
        )
        return tuple(outs)

    devices = jax.devices()[:NCORES]
    mesh = Mesh(np.asarray(devices), ("core",))
    n_outs = len(out_names)
    fn = jax.jit(
        shard_map(
            _body,
            mesh=mesh,
            in_specs=(PartitionSpec("core"),) * (n_params + n_outs),
            out_specs=(PartitionSpec("core"),) * n_outs,
            check_rep=False,
        ),
        keep_unused=True,
    )
    sharding = NamedSharding(mesh, PartitionSpec("core"))
    concat_zeros = [
        jax.device_put(
            np.zeros((NCORES * z.shape[0], *z.shape[1:]), z.dtype), sharding
        )
        for z in zero_outs
    ]
    runner = {
        "fn": fn,
        "in_names": in_names,
        "out_names": out_names,
        "zeros": concat_zeros,
        "sharding": sharding,
    }
    _CACHE[key] = runner
    return runner


def kernel(x, Wq, bq, Wk, bk, Wv, bv):
    import jax

    bq = np.ascontiguousarray(np.asarray(bq, dtype=np.float32))
    bk = np.ascontiguousarray(np.asarray(bk, dtype=np.float32))
    bv_arr = np.ascontiguousarray(np.asarray(bv, dtype=np.float32))
    # the fused path assumes zero biases (scores = X (Wq Wk^T) X^T and V
    # scaled straight from PSUM); fall back to the general path otherwise
    fused = bool(not bq.any() and not bk.any() and not bv_arr.any())

    runner = _get_runner(fused)
    in_maps = make_in_maps(x, Wq, bq, Wk, bk, Wv, bv_arr)

    def _run(r):
        args = []
        for nm in r["in_names"]:
            cat = np.ascontiguousarray(
                np.concatenate([np.asarray(m[nm]) for m in in_maps], axis=0)
            )
            args.append(jax.device_put(cat, r["sharding"]))
        outs = r["fn"](*args, *r["zeros"])
        return np.asarray(outs[r["out_names"].index("out")])

    try:
        out = _run(runner)
    except Exception:
        # stale cached executable/buffers (e.g. device session reset
        # between calls): rebuild once and retry
        _CACHE.pop(("runner", fused), None)
        out = _run(_get_runner(fused))
    return out.reshape(B, T, N, F)


# revision 39
# speedup vs baseline: 1.1465x; 1.0286x over previous
"""GAT-style attention-diagonal kernel for Trainium2 (Bass/Tile), 8-core SPMD.

Reference computation (per (b,t) slice, x:[N,F]):
    Q = x@Wq + bq; K = x@Wk + bk; V = x@Wv + bv
    s = Q @ K.T / sqrt(F)            # [N,N]
    a = softmax(s, axis=-1)
    out = diag(a)[:, None] * V       # only the softmax diagonal is needed

Sharding: data-parallel on the fused B*T axis (48 slices -> 6 per core),
weights replicated.

v4 dataflow (fused path, bq=bk=bv=0), mixed precision:
  - host supplies x TRANSPOSED per slice as [fi, fo, n] in BOTH bf16 and
    fp8(e4m3) (pure layout/dtype prep, like the sharding reshape); this
    removes the on-device transposes, the f32->bf16 pass and both
    PSUM->SBUF transpose copies -- the dominant cross-engine feed chain --
    and frees 2 PSUM banks so the score accumulators double-buffer.
  - the N x N score matrix is only needed for the softmax DENOMINATOR
    (row sums of exp, ~1024 terms): fp8 inputs give ~0.5% denominator
    error.  The score matmul runs in fp8 DoubleRow mode (2 contraction
    rows per PE pass = 2x bf16 throughput).
  - the diagonal s_nn (which sets output accuracy) is recomputed exactly
    per 128-row chunk as a [128,128] bf16 matmul block, diagonal
    extracted by identity mask on DVE.
  - V projection and the one-time-fused XM projection (M = Wq @ Wk.T,
    eliminating the K projection) run in bf16.
  - error budget: bf16 X/M/V ~0.2%, fp8 denominator ~0.5% against the
    2e-2 harness tolerance.
"""

import numpy as np

B, T, N, F = 4, 12, 1024, 512
NCORES = 8
S = (B * T) // NCORES  # 6 slices per core
P = 128
NO = N // P   # 8 row chunks per slice
FO = F // P   # 4 f chunks
GO = F // P   # 4 g chunks
MH = N // 512  # 2 halves of the scores free axis
SCALE = float(1.0 / np.sqrt(np.float32(F)))

_CACHE: dict = {}


def _np_dtypes():
    from concourse import mybir

    return mybir.dt.np(mybir.dt.bfloat16), mybir.dt.np(mybir.dt.float8e4)


def prep_xt(x):
    """Full x [B,T,N,F] (or [BT,N,F]) -> per-slice transposed [BT,P,FO,N]
    in bf16 and fp8e4.  Pure layout/dtype host prep."""
    bf16_t, fp8_t = _np_dtypes()
    x = np.asarray(x, np.float32).reshape(B * T, N, F)
    # A[s, fi, fo, n] = x[s, n, fo*P + fi]
    x4 = np.ascontiguousarray(x.reshape(B * T, N, FO, P).transpose(0, 3, 2, 1))
    return x4.astype(bf16_t), x4.astype(fp8_t)


def build_program(
    n_slices: int = S,
    repeats: int = 1,
    fused_qk: bool = True,
    use_for_i: bool = False,
    pe_only: bool = False,
    lvl: int = 3,
):
    if pe_only:
        lvl = 0
    import concourse.bass as bass
    import concourse.tile as tile
    from concourse import bacc, mybir
    from concourse.masks import make_identity
    from contextlib import ExitStack

    f32 = mybir.dt.float32
    bf16 = mybir.dt.bfloat16
    fp8 = mybir.dt.float8e4
    DR = mybir.MatmulPerfMode.DoubleRow
    EXP = mybir.ActivationFunctionType.Exp
    COPYF = mybir.ActivationFunctionType.Identity
    AX = mybir.AxisListType.X
    OP = mybir.AluOpType

    nc = bacc.Bacc(trn_type="TRN2", target_bir_lowering=False, debug=False)
    xtbf_d = nc.dram_tensor(
        "xtbf", [n_slices, P, FO, N], bf16, kind="ExternalInput"
    ).ap()
    xt8_d = nc.dram_tensor(
        "xt8", [n_slices, P, FO, N], fp8, kind="ExternalInput"
    ).ap()
    wq_d = nc.dram_tensor("wq", [F, F], f32, kind="ExternalInput").ap()
    wk_d = nc.dram_tensor("wk", [F, F], f32, kind="ExternalInput").ap()
    wv_d = nc.dram_tensor("wv", [F, F], f32, kind="ExternalInput").ap()
    bq_d = nc.dram_tensor("bq", [F], f32, kind="ExternalInput").ap()
    bk_d = nc.dram_tensor("bk", [F], f32, kind="ExternalInput").ap()
    bv_d = nc.dram_tensor("bv", [F], f32, kind="ExternalInput").ap()
    out_d = nc.dram_tensor("out", [n_slices, N, F], f32, kind="ExternalOutput").ap()

    with tile.TileContext(nc) as tc, ExitStack() as ctx:
        consts = ctx.enter_context(tc.tile_pool(name="consts", bufs=1))
        stage = ctx.enter_context(tc.tile_pool(name="stage", bufs=1))
        xt_pool = ctx.enter_context(tc.tile_pool(name="xt", bufs=2))
        proj_pool = ctx.enter_context(tc.tile_pool(name="proj", bufs=2))
        outp = ctx.enter_context(tc.tile_pool(name="outp", bufs=3))
        dscr = ctx.enter_context(tc.tile_pool(name="dscr", bufs=2))
        stats = ctx.enter_context(tc.tile_pool(name="stats", bufs=6))
        # PSUM budget: 8 banks = pp(1tag x2) + sp(2tags x2) + dp(1tag x2)
        pp = ctx.enter_context(tc.tile_pool(name="pp", bufs=2, space="PSUM"))
        sp = ctx.enter_context(tc.tile_pool(name="sp", bufs=2, space="PSUM"))
        dp = ctx.enter_context(tc.tile_pool(name="dp", bufs=2, space="PSUM"))

        ident = consts.tile([P, P], f32, name="ident", tag="ident")
        make_identity(nc, ident[:])

        g = {}
        if lvl == 0:
            for nm, shape, dt_ in (
                ("g_xmtbf", [P, GO, N], bf16),
                ("g_xmt8", [P, GO, N], fp8),
            ):
                t = consts.tile(shape, dt_, name=nm, tag=nm)
                nc.vector.memset(t[:], 0)
                g[nm] = t

        def load_xt(s):
            """DMA the host-pretransposed slice: bf16 + fp8, per-fo chunks
            so the XM accumulation starts as soon as fo=0 lands."""
            xt_bf = xt_pool.tile([P, FO, N], bf16, name="xt_bf", tag="xt_bf")
            xt_8 = xt_pool.tile([P, FO, N], fp8, name="xt_8", tag="xt_8")
            for fo in range(0, FO, 2):
                nc.sync.dma_start(xt_bf[:, fo : fo + 2], xtbf_d[s, :, fo : fo + 2])
            nc.sync.dma_start(xt_8[:], xt8_d[s])
            return xt_bf, xt_8

        slice_list = [sl for _ in range(repeats) for sl in range(n_slices)]

        # emit slice 0's loads first so compute starts under weight staging
        xt_first = None
        if not use_for_i:
            xt_first = load_xt(slice_list[0])

        # weights staged as f32
        w_stages = {}
        for nm, wd in (("wq", wq_d), ("wk", wk_d), ("wv", wv_d)):
            w_stage = stage.tile([P, FO, F], f32, name=f"{nm}_stage", tag=f"{nm}_stage")
            nc.sync.dma_start(w_stage[:], wd.rearrange("(fo fi) g -> fi fo g", fi=P))
            w_stages[nm] = w_stage

        wv_bf = consts.tile([P, FO, F], bf16, name="wv_bf", tag="wv_bf")
        nc.vector.tensor_copy(wv_bf[:], w_stages["wv"][:])

        # biases (general path only)
        bq_sb = consts.tile([P, GO], f32, name="bq_sb", tag="bq_sb")
        nc.sync.dma_start(bq_sb[:], bq_d.rearrange("(go gi) -> gi go", gi=P))
        bk_sb = consts.tile([P, GO], f32, name="bk_sb", tag="bk_sb")
        nc.sync.dma_start(bk_sb[:], bk_d.rearrange("(go gi) -> gi go", gi=P))
        bv_bc = consts.tile([P, F], f32, name="bv_bc", tag="bv_bc")
        nc.sync.dma_start(bv_bc[:], bv_d.unsqueeze(0).to_broadcast((P, F)))

        if fused_qk:
            # one-time M = Wq @ Wk.T, stored bf16 like a weight [ai, ao, b].
            # f32 PE transposes through the dp pool's [P, P] bank.
            wt_sbs = {}
            for nm in ("wq", "wk"):
                wt_sb = consts.tile([P, FO, F], bf16, name=f"{nm}t_sb", tag=f"{nm}t_sb")
                for ao in range(FO):
                    for co in range(FO):
                        t_ps = dp.tile([P, P], f32, name="dps", tag="dps")
                        nc.tensor.transpose(
                            t_ps[:],
                            w_stages[nm][:, ao, co * P : (co + 1) * P],
                            ident[:],
                        )
                        nc.vector.tensor_copy(
                            wt_sb[:, co, ao * P : (ao + 1) * P], t_ps[:]
                        )
                wt_sbs[nm] = wt_sb
            m_bf = consts.tile([P, FO, F], bf16, name="m_bf", tag="m_bf")
            for ao in range(FO):
                ps = pp.tile([P, F], f32, name="ps_proj", tag="ps_proj")
                for co in range(FO):
                    nc.tensor.matmul(
                        ps[:],
                        wt_sbs["wq"][:, co, ao * P : (ao + 1) * P],
                        wt_sbs["wk"][:, co, :],
                        start=(co == 0),
                        stop=(co == FO - 1),
                    )
                nc.vector.tensor_copy(m_bf[:, ao, :], ps[:])
        else:
            w_sbs = {}
            for nm in ("wq", "wk"):
                w_sb = consts.tile([P, FO, F], bf16, name=f"{nm}_sb", tag=f"{nm}_sb")
                nc.vector.tensor_copy(w_sb[:], w_stages[nm][:])
                w_sbs[nm] = w_sb

        def emit_slice_fused(s, xt_pair):
            xt_bf, xt_8 = xt_pair if xt_pair is not None else load_xt(s)

            # XMT[b, n] = sum_a M[a, b] * XT[a, n]  -> bf16 + fp8 copies
            if lvl == 0:
                xmt_bf, xmt_8 = g["g_xmtbf"], g["g_xmt8"]
            else:
                xmt_bf = proj_pool.tile([P, GO, N], bf16, name="xmt_bf", tag="xmt_bf")
                xmt_8 = proj_pool.tile([P, GO, N], fp8, name="xmt_8", tag="xmt_8")
            for go in range(GO):
                for nh in range(MH):
                    hsl = slice(nh * 512, (nh + 1) * 512)
                    ps = pp.tile([P, 512], f32, name="ps_proj", tag="ps_proj")
                    for fo in range(FO):
                        nc.tensor.matmul(
                            ps[:],
                            m_bf[:, fo, go * P : (go + 1) * P],
                            xt_bf[:, fo, hsl],
                            start=(fo == 0),
                            stop=(fo == FO - 1),
                        )
                    if lvl >= 1:
                        nc.scalar.activation(xmt_bf[:, go, hsl], ps[:], COPYF)
                        nc.vector.tensor_copy(xmt_8[:, go, hsl], ps[:])

            out_r = out_d[s].rearrange("(g p) f -> p g f", p=P)

            def finish_chunk(m, ps_v, s0, s1, snn, ot2):
                # tail for chunk m, emitted during chunk m+1: keeps the
                # dval chain off the next chunk's critical path
                s01 = stats.tile([P, 1], f32, name="s01", tag="s01")
                nc.vector.tensor_add(s01[:], s0[:], s1[:])
                esnn = stats.tile([P, 1], f32, name="esnn", tag="esnn")
                nc.scalar.activation(esnn[:], snn[:], EXP, scale=SCALE)
                rec = stats.tile([P, 1], f32, name="rec", tag="rec")
                nc.vector.reciprocal(rec[:], s01[:])
                dval = stats.tile([P, 1], f32, name="dval", tag="dval")
                nc.vector.tensor_mul(dval[:], esnn[:], rec[:])
                nc.vector.tensor_scalar_mul(ot2[:, m % 2], ps_v[:], dval[:])
                if m % 2 == 1:
                    nc.sync.dma_start(out_r[:, m - 1 : m + 1], ot2[:])

            prev = None
            ot2 = None
            for no in range(NO):
                nsl = slice(no * P, (no + 1) * P)
                # exact-ish diagonal block (bf16) and V first: they cover
                # the xmt_8 feed latency at the XMT/chunk-loop boundary
                dps = dp.tile([P, P], f32, name="dps", tag="dps")
                for go in range(GO):
                    nc.tensor.matmul(
                        dps[:],
                        xmt_bf[:, go, nsl],
                        xt_bf[:, go, nsl],
                        start=(go == 0),
                        stop=(go == GO - 1),
                    )
                # V row-chunk (bf16), scaled straight from PSUM in the tail
                ps_v = pp.tile([P, F], f32, name="ps_proj", tag="ps_proj")
                for fo in range(FO):
                    nc.tensor.matmul(
                        ps_v[:],
                        xt_bf[:, fo, nsl],
                        wv_bf[:, fo, :],
                        start=(fo == 0),
                        stop=(fo == FO - 1),
                    )
                # scores row-chunk in fp8 DoubleRow: 2 k-pair passes per bank
                ps0 = sp.tile([P, 512], f32, name="ps_s0", tag="ps_s0")
                ps1 = sp.tile([P, 512], f32, name="ps_s1", tag="ps_s1")
                for j in range(2):
                    ksl = slice(2 * j, 2 * j + 2)
                    lhsT = xmt_8[:, ksl, nsl]
                    nc.tensor.matmul(
                        ps0[:], lhsT, xt_8[:, ksl, 0:512],
                        start=(j == 0), stop=(j == 1), perf_mode=DR,
                    )
                    nc.tensor.matmul(
                        ps1[:], lhsT, xt_8[:, ksl, 512:1024],
                        start=(j == 0), stop=(j == 1), perf_mode=DR,
                    )

                if lvl < 2:
                    continue
                s0 = stats.tile([P, 1], f32, name="s0", tag="s0")
                s1 = stats.tile([P, 1], f32, name="s1", tag="s1")
                nc.scalar.activation(ps0[:], ps0[:], EXP, scale=SCALE, accum_out=s0[:])
                nc.scalar.activation(ps1[:], ps1[:], EXP, scale=SCALE, accum_out=s1[:])
                if lvl < 3:
                    continue
                dblk = dscr.tile([P, P], f32, name="dblk", tag="dblk")
                nc.vector.tensor_mul(dblk[:], dps[:], ident[:])
                snn = stats.tile([P, 1], f32, name="snn", tag="snn")
                nc.vector.tensor_reduce(snn[:], dblk[:], axis=AX, op=OP.add)

                if no % 2 == 0:
                    ot2 = outp.tile([P, 2, F], f32, name="ot2", tag="ot2")
                if prev is not None:
                    finish_chunk(*prev)
                prev = (no, ps_v, s0, s1, snn, ot2)
            if lvl >= 3:
                finish_chunk(*prev)

        def emit_slice_general(s):
            """bf16 path with biases (reference-faithful Q/K projections)."""
            xt_bf, _ = load_xt(s)
            v_sb = proj_pool.tile([P, NO, F], f32, name="v_sb", tag="v_sb")
            qt_sb = proj_pool.tile([P, GO, N], bf16, name="qt_sb", tag="qt_sb")
            kt_sb = proj_pool.tile([P, GO, N], bf16, name="kt_sb", tag="kt_sb")
            for w_sb, b_sb, dst in (
                (w_sbs["wq"], bq_sb, qt_sb),
                (w_sbs["wk"], bk_sb, kt_sb),
            ):
                for go in range(GO):
                    for nh in range(MH):
                        ps = pp.tile([P, 512], f32, name="ps_proj", tag="ps_proj")
                        for fo in range(FO):
                            nc.tensor.matmul(
                                ps[:],
                                w_sb[:, fo, go * P : (go + 1) * P],
                                xt_bf[:, fo, nh * 512 : (nh + 1) * 512],
                                start=(fo == 0),
                                stop=(fo == FO - 1),
                            )
                        nc.vector.tensor_scalar_add(
                            dst[:, go, nh * 512 : (nh + 1) * 512],
                            ps[:],
                            b_sb[:, go : go + 1],
                        )
            for no in range(NO):
                ps = pp.tile([P, F], f32, name="ps_proj", tag="ps_proj")
                for fo in range(FO):
                    nc.tensor.matmul(
                        ps[:],
                        xt_bf[:, fo, no * P : (no + 1) * P],
                        wv_bf[:, fo, :],
                        start=(fo == 0),
                        stop=(fo == FO - 1),
                    )
                nc.vector.tensor_add(v_sb[:, no, :], ps[:], bv_bc[:])

            for no in range(NO):
                ps0 = sp.tile([P, 512], f32, name="ps_s0", tag="ps_s0")
                ps1 = sp.tile([P, 512], f32, name="ps_s1", tag="ps_s1")
                for go in range(GO):
                    lhsT = qt_sb[:, go, no * P : (no + 1) * P]
                    nc.tensor.matmul(
                        ps0[:], lhsT, kt_sb[:, go, 0:512],
                        start=(go == 0), stop=(go == GO - 1),
                    )
                    nc.tensor.matmul(
                        ps1[:], lhsT, kt_sb[:, go, 512:1024],
                        start=(go == 0), stop=(go == GO - 1),
                    )
                s0 = stats.tile([P, 1], f32, name="s0", tag="s0")
                s1 = stats.tile([P, 1], f32, name="s1", tag="s1")
                nc.scalar.activation(ps0[:], ps0[:], EXP, scale=SCALE, accum_out=s0[:])
                nc.scalar.activation(ps1[:], ps1[:], EXP, scale=SCALE, accum_out=s1[:])
                bank, off = divmod(no * P, 512)
                psd = ps0 if bank == 0 else ps1
                dblk = dscr.tile([P, P], f32, name="dblk", tag="dblk")
                nc.vector.tensor_mul(dblk[:], psd[:, off : off + P], ident[:])
                snn = stats.tile([P, 1], f32, name="snn", tag="snn")
                nc.vector.tensor_reduce(snn[:], dblk[:], axis=AX, op=OP.add)
                ssum = stats.tile([P, 1], f32, name="ssum", tag="ssum")
                nc.vector.tensor_add(ssum[:], s0[:], s1[:])
                rec = stats.tile([P, 1], f32, name="rec", tag="rec")
                nc.vector.reciprocal(rec[:], ssum[:])
                dval = stats.tile([P, 1], f32, name="dval", tag="dval")
                nc.vector.tensor_mul(dval[:], snn[:], rec[:])
                ot = outp.tile([P, F], f32, name="ot", tag="ot")
                nc.vector.tensor_scalar_mul(ot[:], v_sb[:, no, :], dval[:])
                nc.sync.dma_start(out_d[s, no * P : (no + 1) * P, :], ot[:])

        if use_for_i:
            with tc.For_i(0, repeats) as _i:
                for s in range(n_slices):
                    if fused_qk:
                        emit_slice_fused(s, None)
                    else:
                        emit_slice_general(s)
        else:
            for i, s in enumerate(slice_list):
                if fused_qk:
                    emit_slice_fused(s, xt_first if i == 0 else None)
                else:
                    emit_slice_general(s)

    nc.compile()
    return nc


def make_in_maps(x, Wq, bq, Wk, bk, Wv, bv):
    """Per-core input dicts for the Bass program (host does the transpose
    + bf16/fp8 casts)."""
    xtbf, xt8 = prep_xt(x)
    xtbf = xtbf.reshape(NCORES, S, P, FO, N)
    xt8 = xt8.reshape(NCORES, S, P, FO, N)
    common = {
        "wq": np.asarray(Wq, np.float32),
        "wk": np.asarray(Wk, np.float32),
        "wv": np.asarray(Wv, np.float32),
        "bq": np.asarray(bq, np.float32),
        "bk": np.asarray(bk, np.float32),
        "bv": np.asarray(bv, np.float32),
    }
    return [dict(common, xtbf=xtbf[c], xt8=xt8[c]) for c in range(NCORES)]


def _get_runner(fused: bool):
    """Build the Bass program once and wrap it in a cached jitted shard_map
    dispatcher (mirrors bass2jax.run_bass_via_pjrt, minus donation so the
    pre-zeroed output operands can be reused across calls — this kernel
    writes every output element)."""
    key = ("runner", fused)
    if key in _CACHE:
        return _CACHE[key]

    import jax
    from jax.experimental.shard_map import shard_map
    from jax.sharding import Mesh, NamedSharding, PartitionSpec
    from concourse import mybir
    from concourse.bass2jax import (
        _bass_exec_p,
        install_neuronx_cc_hook,
        partition_id_tensor,
    )

    nc = build_program(S, fused_qk=fused)
    install_neuronx_cc_hook()
    partition_name = nc.partition_id_tensor.name if nc.partition_id_tensor else None

    in_names, out_names, out_avals, zero_outs = [], [], [], []
    for alloc in nc.m.functions[0].allocations:
        if not isinstance(alloc, mybir.MemoryLocationSet):
            continue
        name = alloc.memorylocations[0].name
        if alloc.kind == "ExternalInput":
            if name != partition_name:
                in_names.append(name)
        elif alloc.kind == "ExternalOutput":
            shape = tuple(alloc.tensor_shape)
            np_dt = mybir.dt.np(alloc.dtype)
            out_avals.append(jax.core.ShapedArray(shape, np_dt))
            out_names.append(name)
            zero_outs.append(np.zeros(shape, np_dt))

    n_params = len(in_names)
    all_in_names = list(in_names) + list(out_names)
    if partition_name is not None:
        all_in_names.append(partition_name)

    def _body(*args):
        operands = list(args)
        if partition_name is not None:
            operands.append(partition_id_tensor())
        outs = _bass_exec_p.bind(
            *operands,
            out_avals=tuple(out_avals),
            in_names=tuple(all_in_names),
            out_names=tuple(out_names),
            lowering_input_output_aliases=(),
            sim_require_finite=True,
            sim_require_nnan=True,
            nc=nc,
        )
        return tuple(outs)

    devices = jax.devices()[:NCORES]
    mesh = Mesh(np.asarray(devices), ("core",))
    n_outs = len(out_names)
    fn = jax.jit(
        shard_map(
            _body,
            mesh=mesh,
            in_specs=(PartitionSpec("core"),) * (n_params + n_outs),
            out_specs=(PartitionSpec("core"),) * n_outs,
            check_rep=False,
        ),
        keep_unused=True,
    )
    sharding = NamedSharding(mesh, PartitionSpec("core"))
    concat_zeros = [
        jax.device_put(
            np.zeros((NCORES * z.shape[0], *z.shape[1:]), z.dtype), sharding
        )
        for z in zero_outs
    ]
    runner = {
        "fn": fn,
        "in_names": in_names,
        "out_names": out_names,
        "zeros": concat_zeros,
        "sharding": sharding,
    }
    _CACHE[key] = runner
    return runner


def kernel(x, Wq, bq, Wk, bk, Wv, bv):
    import jax

    bq = np.ascontiguousarray(np.asarray(bq, dtype=np.float32))
    bk = np.ascontiguousarray(np.asarray(bk, dtype=np.float32))
    bv_arr = np.ascontiguousarray(np.asarray(bv, dtype=np.float32))
    # the fused path assumes zero biases (scores = X (Wq Wk^T) X^T and V
    # scaled straight from PSUM); fall back to the general path otherwise
    fused = bool(not bq.any() and not bk.any() and not bv_arr.any())

    runner = _get_runner(fused)
    in_maps = make_in_maps(x, Wq, bq, Wk, bk, Wv, bv_arr)

    def _run(r):
        args = []
        for nm in r["in_names"]:
            cat = np.ascontiguousarray(
                np.concatenate([np.asarray(m[nm]) for m in in_maps], axis=0)
            )
            args.append(jax.device_put(cat, r["sharding"]))
        outs = r["fn"](*args, *r["zeros"])
        return np.asarray(outs[r["out_names"].index("out")])

    try:
        out = _run(runner)
    except Exception:
        # stale cached executable/buffers (e.g. device session reset
        # between calls): rebuild once and retry
        _CACHE.pop(("runner", fused), None)
        out = _run(_get_runner(fused))
    return out.reshape(B, T, N, F)
